# revision 8
# baseline (speedup 1.0000x reference)
"""Causal self-attention (B=2, T=2048, C=1024, H=16) on 8 Trainium2 NeuronCores.

Sharding: data-parallel over batch (2) x tensor-parallel over heads (4 groups
of 4 heads) = 8 cores. c_attn column-sharded, c_proj row-sharded; each core
emits a partial [C, T] projection output (bf16) that the host sums per batch.

All matmuls run in bf16 with fp32 PSUM accumulation. Attention scores are
computed transposed (S^T = K Q^T, k on partitions, two heads row-tiled
concurrently on disjoint PE row groups). The PV matmul keeps V stationary
(65 columns: 64 V dims + a ones column that accumulates the softmax
denominator) and streams P 512 wide. Normalization: DVE reciprocal on the
denominator PSUM row, gpsimd partition_broadcast, DVE multiply.

v2 perf changes vs v1 (baseline 181us):
 - inputs loaded with 7 batched DMA instructions (issue on Sync AND Scalar
   HWDGE queues) instead of 52 serial ~650ns issues on Sync
 - PE warm-up burst of tiny matmuls at t~6us so the HAM clock gate reaches
   K=8/8 (2.4 GHz) before real compute starts
 - softmax 1/denom: no more [1,512]<->[128,4] DMA transpose round trips
   (they were 16384 DMA packets congesting the queues + latency in the PV
   critical path) and no PE broadcast matmul
 - causal masking of diagonal blocks via gpsimd affine_select (idle engine)
   instead of DVE tensor_mul with a tri matrix
 - projection outputs cast to bf16 on DVE (was fp32 copies on the busy
   Scalar engine) and written with one batched DMA per query strip
 - PSUM: qkv/proj pool double-buffered (was 1 bank shared by all, which
   serialized every matmul group behind the previous PSUM drain)
"""

import numpy as np
import ml_dtypes

BF = ml_dtypes.bfloat16

B, T, C, H, DH = 2, 2048, 1024, 16, 64
N_CORES = 8
G = 4            # head groups (tensor-parallel)
HPG = 4          # heads per group
TQ = 512         # query strip width
TK = 128         # key tile width
NSTRIP = T // TQ        # 4 query strips
NKT = T // TK           # 16 key tiles
NCT = C // 128          # 8 contraction tiles for qkv
VST = 136               # V2 per-k-tile stride: 2 heads x (64 V + 1 ones + 3 pad)

_CACHE = {}


def _ensure_runtime():
    """Import jax (boots the axon PJRT plugin) exactly once."""
    import jax
    jax.devices()


def _build(with_bias: bool):
    import concourse.tile as tile
    from concourse import bacc, mybir

    f32 = mybir.dt.float32
    bf16 = mybir.dt.bfloat16
    Exp = mybir.ActivationFunctionType.Exp

    nc = bacc.Bacc("TRN2", target_bir_lowering=False, debug=False,
                   enable_asserts=False, num_devices=N_CORES)

    xT_d = nc.dram_tensor("xT", [C, T], bf16, kind="ExternalInput").ap()
    wqk_d = nc.dram_tensor("wqk", [C, 512], bf16, kind="ExternalInput").ap()
    wv_d = nc.dram_tensor("wv", [C, 256], bf16, kind="ExternalInput").ap()
    wp_d = nc.dram_tensor("wp", [256, C], bf16, kind="ExternalInput").ap()
    if with_bias:
        bqk_d = nc.dram_tensor("bqk", [1, 512], bf16, kind="ExternalInput").ap()
        bv_d = nc.dram_tensor("bv", [1, 256], bf16, kind="ExternalInput").ap()
    out_d = nc.dram_tensor("outT", [C, T], bf16, kind="ExternalOutput").ap()

    with tile.TileContext(nc) as tc:
        with (
            tc.tile_pool(name="persist", bufs=1) as pp,
            tc.tile_pool(name="pP", bufs=34) as pP,
            tc.tile_pool(name="rrow", bufs=6) as pRR,
            tc.tile_pool(name="bcsb", bufs=6) as pBC,
            tc.tile_pool(name="ob", bufs=2) as pO,
            tc.tile_pool(name="psum", bufs=1, space="PSUM") as psp,
        ):
            # ---- persistent SBUF tensors -------------------------------
            xTall = pp.tile([128, NCT * T], bf16, tag="xTall", name="xTall")
            xT = [xTall[:, i * T:(i + 1) * T] for i in range(NCT)]
            wqkall = pp.tile([128, NCT * 512], bf16, tag="wqkall", name="wqkall")
            wqk = [wqkall[:, i * 512:(i + 1) * 512] for i in range(NCT)]
            wvall = pp.tile([128, NCT * 256], bf16, tag="wvall", name="wvall")
            wv = [wvall[:, i * 256:(i + 1) * 256] for i in range(NCT)]
            wpall = pp.tile([128, 2 * C], bf16, tag="wpall", name="wpall")
            wp = [wpall[:, p * C:(p + 1) * C] for p in range(2)]
            QTp = [pp.tile([128, T], bf16, tag=f"QT{p}", name=f"QT{p}")
                   for p in range(2)]
            KTp = [pp.tile([128, T], bf16, tag=f"KT{p}", name=f"KT{p}")
                   for p in range(2)]
            V2 = [pp.tile([128, NKT * VST], bf16, tag=f"V{p}", name=f"V{p}")
                  for p in range(2)]
            yT2 = [pp.tile([128, T], bf16, tag=f"yT{p}", name=f"yT{p}")
                   for p in range(2)]
            wseed = pp.tile([1, 64], bf16, tag="wseed", name="wseed")
            if with_bias:
                bqk = pp.tile([1, 512], bf16, tag="bqk", name="bqk")
                bv = pp.tile([1, 256], bf16, tag="bv", name="bv")
                ones_row = pp.tile([1, 512], bf16, tag="ones", name="ones")

            # ---- PE warm-up: tiny matmuls flip the HAM clock gate to
            # 2.4 GHz while the input DMAs stream in ----------------------
            nc.gpsimd.memset(wseed[:], 0.125)
            warm_ps = psp.tile([65, TQ], f32, tag="pv", bufs=2, name="warmps")
            for _ in range(72):
                nc.tensor.matmul(warm_ps[0:64, 0:64], lhsT=wseed[:],
                                 rhs=wseed[:], start=True, stop=True)

            # ---- batched input DMAs ------------------------------------
            # ~256-512KB chunks so transfers parallelize across queues while
            # keeping issue count low. First-needed data (wqk + xT strip 0)
            # leads on both HWDGE issue queues (Sync and Scalar).
            xT_d3 = xT_d.rearrange("(i p) t -> p i t", p=128)
            xTall3 = xTall[:, :].rearrange("p (i t) -> p i t", t=T)
            wqk_d3 = wqk_d.rearrange("(i p) m -> p i m", p=128)
            wqkall3 = wqkall[:, :].rearrange("p (i m) -> p i m", m=512)

            def x_chunk(eng, c, i0, i1):
                eng.dma_start(
                    xTall3[:, i0:i1, c * TQ:(c + 1) * TQ],
                    xT_d3[:, i0:i1, c * TQ:(c + 1) * TQ])

            # scalar queue: wqk (4 chunks) then strip1, wp
            for i0 in range(0, 8, 2):
                nc.scalar.dma_start(wqkall3[:, i0:i0 + 2, :],
                                    wqk_d3[:, i0:i0 + 2, :])
            # sync queue: strip0 (4 chunks), wv, then strips 2-3
            for i0 in range(0, 8, 2):
                x_chunk(nc.sync, 0, i0, i0 + 2)
            nc.sync.dma_start(
                wvall[:, :].rearrange("p (i m) -> p i m", m=256),
                wv_d.rearrange("(i p) m -> p i m", p=128))
            for i0 in range(0, 8, 4):
                x_chunk(nc.scalar, 1, i0, i0 + 4)
            nc.scalar.dma_start(
                wpall[:, :].rearrange("p (i m) -> p i m", m=C),
                wp_d.rearrange("(i p) m -> p i m", p=128))
            for i0 in range(0, 8, 4):
                x_chunk(nc.sync, 2, i0, i0 + 4)
            for i0 in range(0, 8, 4):
                x_chunk(nc.sync, 3, i0, i0 + 4)
            if with_bias:
                nc.scalar.dma_start(bqk[:], bqk_d[:, :])
                nc.scalar.dma_start(bv[:], bv_d[:, :])
                nc.gpsimd.memset(ones_row[:], 1.0)
            # ones columns for the softmax denominator: memset the whole V2
            # tiles to 1.0 once; the V columns get overwritten by v_steps.
            for p in range(2):
                nc.gpsimd.memset(V2[p][:], 1.0)

            nbias = 1 if with_bias else 0
            P_store = {}

            def A_steps(pair, qt):
                """Step list: each step emits one complete psum group."""
                steps = []

                def qk_step(mt):
                    def f():
                        ps = psp.tile([128, TQ], f32, tag="big", bufs=2, name="psA")
                        for ci in range(NCT):
                            nc.tensor.matmul(
                                ps[:],
                                lhsT=wqk[ci][:, mt * 128:(mt + 1) * 128],
                                rhs=xT[ci][:, qt * TQ:(qt + 1) * TQ],
                                start=(ci == 0), stop=(ci == NCT + nbias - 1))
                        if with_bias:
                            nc.tensor.matmul(
                                ps[:], lhsT=bqk[0:1, mt * 128:(mt + 1) * 128],
                                rhs=ones_row[0:1, 0:TQ], start=False, stop=True)
                        dst = QTp[pair] if mt < 2 else KTp[pair]
                        nc.vector.tensor_copy(dst[:, qt * TQ:(qt + 1) * TQ], ps[:])
                    return f

                def v_step(kt):
                    def f():
                        psv = psp.tile([128, 256], f32, tag="big", bufs=2, name="psVt")
                        for ci in range(NCT):
                            nc.tensor.matmul(
                                psv[:],
                                lhsT=xT[ci][:, kt * 128:(kt + 1) * 128],
                                rhs=wv[ci][:, :],
                                start=(ci == 0), stop=(ci == NCT + nbias - 1))
                        if with_bias:
                            nc.tensor.matmul(
                                psv[:], lhsT=ones_row[0:1, 0:128], rhs=bv[0:1, :],
                                start=False, stop=True)
                        for p in range(2):
                            s3 = psv[:, p * 128:(p + 1) * 128] \
                                .rearrange("q (a b) -> q a b", b=64)
                            d3 = V2[p][:, kt * VST: kt * VST + VST] \
                                .rearrange("q (a b) -> q a b", b=68)[:, :, 0:64]
                            nc.vector.tensor_copy(d3, s3)
                    return f

                for mt in (pair, 2 + pair):
                    steps.append(qk_step(mt))
                if pair == 1:
                    for kt in range(4 * qt, 4 * qt + 4):
                        steps.append(v_step(kt))
                return steps

            def S_steps(pair, qt):
                """One step per k-tile: paired QK matmuls (disjoint PE row
                groups) + the two exps."""
                nk = 4 * (qt + 1)

                def kt_step(kt):
                    def f():
                        ps = psp.tile([128, 2 * TQ], f32, tag="S", bufs=2, name="psS")
                        m = kt - 4 * qt
                        off = max(0, m) * 128
                        for hh in range(2):
                            nc.tensor.matmul(
                                ps[:, hh * TQ + off:(hh + 1) * TQ],
                                lhsT=KTp[pair][hh * 64:(hh + 1) * 64,
                                               kt * 128:(kt + 1) * 128],
                                rhs=QTp[pair][hh * 64:(hh + 1) * 64,
                                              qt * TQ + off:(qt + 1) * TQ],
                                start=True, stop=True)
                        Pt = pP.tile([128, 2 * TQ], bf16, tag="P", bufs=32, name="Pt")
                        if m < 0:    # one exp across both heads' banks
                            nc.scalar.activation(Pt[:, :], ps[:, :], Exp, scale=0.125)
                        else:        # one 3D-AP exp covering both heads
                            nc.scalar.activation(
                                Pt[:, :].rearrange("k (h q) -> k h q", h=2)
                                        [:, :, off:TQ],
                                ps[:, :].rearrange("k (h q) -> k h q", h=2)
                                        [:, :, off:TQ],
                                Exp, scale=0.125)
                        for hh in range(2):
                            h = 2 * pair + hh
                            if m >= 0:   # diagonal block: zero k > q (gpsimd)
                                sl = Pt[:, hh * TQ + off: hh * TQ + off + 128]
                                nc.gpsimd.affine_select(
                                    out=sl, in_=sl,
                                    compare_op=mybir.AluOpType.is_ge,
                                    fill=0.0, base=0, pattern=[[1, 128]],
                                    channel_multiplier=-1)
                            P_store[(h, qt, kt)] = Pt[:, hh * TQ:(hh + 1) * TQ]
                    return f

                return [kt_step(kt) for kt in range(nk)]

            def PV_steps(pair, qt):
                """One step per head: PV accumulation (V stationary), then
                1/denom on DVE, partition-broadcast on gpsimd, multiply."""
                nk = 4 * (qt + 1)

                def head_step(hh):
                    def f():
                        h = 2 * pair + hh
                        psy = psp.tile([65, TQ], f32, tag="pv", bufs=2, name="psy")
                        for kt in range(nk):
                            off = max(0, kt - 4 * qt) * 128
                            nc.tensor.matmul(
                                psy[:, off:TQ],
                                lhsT=V2[pair][:, kt * VST + hh * 68:
                                              kt * VST + hh * 68 + 65],
                                rhs=P_store[(h, qt, kt)][:, off:TQ],
                                start=(kt == 0), stop=(kt == nk - 1))
                        drow = pRR.tile([1, TQ], f32, tag="rr", name="drow")
                        nc.vector.tensor_copy(drow[:], psy[64:65, :])
                        rfast = pRR.tile([1, TQ], f32, tag="rrb", name="rfast")
                        nc.vector.reciprocal_approx_fast(rfast[:], drow[:])
                        bcs = pBC.tile([64, TQ], f32, tag="bc", name="bcs")
                        nc.gpsimd.partition_broadcast(bcs[:], rfast[0:1, :],
                                                      channels=64)
                        nc.vector.tensor_mul(
                            yT2[pair][hh * 64:(hh + 1) * 64, qt * TQ:(qt + 1) * TQ],
                            psy[0:64, :], bcs[:])
                        if hh == 1:
                            for kt in range(nk):
                                del P_store[(2 * pair, qt, kt)]
                                del P_store[(2 * pair + 1, qt, kt)]
                    return f

                return [head_step(0), head_step(1)]

            def PJ_steps(qt):
                yOut = pO.tile([128, 8 * TQ], bf16, tag="yout", bufs=2,
                               name=f"yOut{qt}")

                def co_step(co):
                    def f():
                        pso = psp.tile([128, TQ], f32, tag="big", bufs=2, name="psO")
                        for pair in range(2):
                            nc.tensor.matmul(
                                pso[:],
                                lhsT=wp[pair][:, co * 128:(co + 1) * 128],
                                rhs=yT2[pair][:, qt * TQ:(qt + 1) * TQ],
                                start=(pair == 0), stop=(pair == 1))
                        nc.vector.tensor_copy(
                            yOut[:, co * TQ:(co + 1) * TQ], pso[:])
                        if co == 7:
                            nc.sync.dma_start(
                                out_d.rearrange("(i p) t -> p i t", p=128)
                                     [:, :, qt * TQ:(qt + 1) * TQ],
                                yOut[:, :].rearrange("p (i t) -> p i t", t=TQ))
                    return f

                return [co_step(co) for co in range(8)]

            def weave(s_list, others):
                """Interleave `others` proportionally between S k-tile steps."""
                if not s_list:
                    for f in others:
                        f()
                    return
                r = len(others) / len(s_list)
                acc, oi = 0.5, 0
                for f in s_list:
                    f()
                    acc += r
                    while acc >= 1.0 and oi < len(others):
                        others[oi]()
                        oi += 1
                        acc -= 1.0
                while oi < len(others):
                    others[oi]()
                    oi += 1

            # ---- software-pipelined, finely woven emission order -------
            # A steps run as early as their xT strip allows; each PV phase
            # follows its S phase one weave-line later so P tiles die fast
            # and the last S phase (pair 1, qt 3) overlaps PV(0,3)+PJ(2).
            for f in A_steps(0, 0):
                f()
            weave(S_steps(0, 0), A_steps(1, 0))
            weave(S_steps(1, 0), A_steps(0, 1) + PV_steps(0, 0))
            weave(S_steps(0, 1), A_steps(1, 1) + PV_steps(1, 0))
            weave(S_steps(1, 1), A_steps(0, 2) + PV_steps(0, 1) + PJ_steps(0))
            weave(S_steps(0, 2), A_steps(1, 2) + PV_steps(1, 1))
            weave(S_steps(1, 2), A_steps(0, 3) + PV_steps(0, 2) + PJ_steps(1))
            weave(S_steps(0, 3), A_steps(1, 3) + PV_steps(1, 2))
            weave(S_steps(1, 3), PV_steps(0, 3) + PJ_steps(2))
            weave([], PV_steps(1, 3) + PJ_steps(3))

    nc.compile()
    return nc


def _get_nc(with_bias: bool):
    key = ("nc", with_bias)
    if key not in _CACHE:
        _ensure_runtime()
        _CACHE[key] = _build(with_bias)
    return _CACHE[key]


def _shard_inputs(x, w_qkv, b_qkv, w_proj, with_bias):
    """Build the 8 per-core input maps (bf16)."""
    in_maps = []
    for core in range(N_CORES):
        b, g = core // G, core % G
        hs = [g * HPG + i for i in range(HPG)]
        q_cols = [w_qkv[:, h * DH:(h + 1) * DH] for h in hs]
        k_cols = [w_qkv[:, C + h * DH: C + (h + 1) * DH] for h in hs]
        v_cols = [w_qkv[:, 2 * C + h * DH: 2 * C + (h + 1) * DH] for h in hs]
        m = {
            "xT": np.ascontiguousarray(x[b].T).astype(BF),
            "wqk": np.concatenate(q_cols + k_cols, axis=1).astype(BF),
            "wv": np.concatenate(v_cols, axis=1).astype(BF),
            "wp": np.concatenate(
                [w_proj[h * DH:(h + 1) * DH, :] for h in hs], axis=0).astype(BF),
        }
        if with_bias:
            bq = [b_qkv[h * DH:(h + 1) * DH] for h in hs]
            bk = [b_qkv[C + h * DH: C + (h + 1) * DH] for h in hs]
            bvs = [b_qkv[2 * C + h * DH: 2 * C + (h + 1) * DH] for h in hs]
            m["bqk"] = np.concatenate(bq + bk)[None, :].astype(BF)
            m["bv"] = np.concatenate(bvs)[None, :].astype(BF)
        in_maps.append(m)
    return in_maps


def run_on_device(x, w_qkv, b_qkv, w_proj, b_proj, trace=False, trace_kwargs=None):
    """Returns (output [B,T,C] float32, BassKernelResults)."""
    x = np.asarray(x, np.float32)
    w_qkv = np.asarray(w_qkv, np.float32)
    b_qkv = np.asarray(b_qkv, np.float32)
    w_proj = np.asarray(w_proj, np.float32)
    b_proj = np.asarray(b_proj, np.float32)

    with_bias = bool(np.any(b_qkv))
    nc = _get_nc(with_bias)
    in_maps = _shard_inputs(x, w_qkv, b_qkv, w_proj, with_bias)

    from concourse.bass_utils import run_bass_kernel_spmd
    res = run_bass_kernel_spmd(nc, in_maps, core_ids=list(range(N_CORES)),
                               trace=trace, **(trace_kwargs or {}))

    out = np.zeros((B, T, C), np.float64)
    for core in range(N_CORES):
        b = core // G
        out[b] += res.results[core]["outT"].T.astype(np.float64)
    out += b_proj.astype(np.float64)[None, None, :]
    return out.astype(np.float32), res


def kernel(x, w_qkv, b_qkv, w_proj, b_proj):
    out, _ = run_on_device(x, w_qkv, b_qkv, w_proj, b_proj)
    return out


# revision 16
# speedup vs baseline: 1.0716x; 1.0716x over previous
"""Causal self-attention (B=2, T=2048, C=1024, H=16) on 8 Trainium2 NeuronCores.

Sharding: data-parallel over batch (2) x tensor-parallel over heads (4 groups
of 4 heads) = 8 cores. c_attn column-sharded, c_proj row-sharded; each core
emits a partial [C, T] projection output (bf16) that the host sums per batch.

All matmuls run in bf16 with fp32 PSUM accumulation. Attention scores are
computed transposed (S^T = K Q^T, k on partitions, two heads row-tiled
concurrently on disjoint PE row groups). The PV matmul keeps V stationary
(65 columns: 64 V dims + a ones column that accumulates the softmax
denominator) and streams P 512 wide. Normalization: DVE reciprocal on the
denominator PSUM row, gpsimd partition_broadcast, DVE multiply.

v2 perf changes vs v1 (baseline 181us):
 - inputs loaded with 7 batched DMA instructions (issue on Sync AND Scalar
   HWDGE queues) instead of 52 serial ~650ns issues on Sync
 - PE warm-up burst of tiny matmuls at t~6us so the HAM clock gate reaches
   K=8/8 (2.4 GHz) before real compute starts
 - softmax 1/denom: no more [1,512]<->[128,4] DMA transpose round trips
   (they were 16384 DMA packets congesting the queues + latency in the PV
   critical path) and no PE broadcast matmul
 - causal masking of diagonal blocks via gpsimd affine_select (idle engine)
   instead of DVE tensor_mul with a tri matrix
 - projection outputs cast to bf16 on DVE (was fp32 copies on the busy
   Scalar engine) and written with one batched DMA per query strip
 - PSUM: qkv/proj pool double-buffered (was 1 bank shared by all, which
   serialized every matmul group behind the previous PSUM drain)
"""

import numpy as np
import ml_dtypes

BF = ml_dtypes.bfloat16
F8 = ml_dtypes.float8_e4m3   # TRN FP8_EXP4-compatible for |v| <= 240
WS = 32.0                    # host pre-scale on wq/wk so fp8 avoids denormals

B, T, C, H, DH = 2, 2048, 1024, 16, 64
N_CORES = 8
G = 4            # head groups (tensor-parallel)
HPG = 4          # heads per group
TQ = 512         # query strip width
TK = 128         # key tile width
NSTRIP = T // TQ        # 4 query strips
NKT = T // TK           # 16 key tiles
NCT = C // 128          # 8 contraction tiles for qkv
VST = 136               # V2 per-k-tile stride: 2 heads x (64 V + 1 ones + 3 pad)

_CACHE = {}


def _ensure_runtime():
    """Import jax (boots the axon PJRT plugin) exactly once."""
    import jax
    jax.devices()


def _build(with_bias: bool):
    import concourse.tile as tile
    from concourse import bacc, mybir

    f32 = mybir.dt.float32
    bf16 = mybir.dt.bfloat16
    fp8 = mybir.dt.float8e4
    Exp = mybir.ActivationFunctionType.Exp
    DoubleRow = mybir.MatmulPerfMode.DoubleRow

    nc = bacc.Bacc("TRN2", target_bir_lowering=False, debug=False,
                   enable_asserts=False, num_devices=N_CORES)

    xT_d = nc.dram_tensor("xT", [C, T], bf16, kind="ExternalInput").ap()
    # fp8 DoubleRow operands, host-prepacked [ki=128, g=4, ko=2, ...]
    x8_d = nc.dram_tensor("x8", [128, 8 * T], fp8, kind="ExternalInput").ap()
    wqk8_d = nc.dram_tensor("wqk8", [128, 8 * 512], fp8, kind="ExternalInput").ap()
    wv_d = nc.dram_tensor("wv", [C, 256], bf16, kind="ExternalInput").ap()
    wp_d = nc.dram_tensor("wp", [256, C], bf16, kind="ExternalInput").ap()
    if with_bias:
        bqk_d = nc.dram_tensor("bqk", [1, 512], bf16, kind="ExternalInput").ap()
        bv_d = nc.dram_tensor("bv", [1, 256], bf16, kind="ExternalInput").ap()
    out_d = nc.dram_tensor("outT", [C, T], bf16, kind="ExternalOutput").ap()

    with tile.TileContext(nc) as tc:
        with (
            tc.tile_pool(name="persist", bufs=1) as pp,
            tc.tile_pool(name="pP", bufs=34) as pP,
            tc.tile_pool(name="rrow", bufs=3) as pRR,
            tc.tile_pool(name="bcsb", bufs=4) as pBC,
            tc.tile_pool(name="ob", bufs=2) as pO,
            tc.tile_pool(name="psum", bufs=1, space="PSUM") as psp,
        ):
            # ---- persistent SBUF tensors -------------------------------
            xTall = pp.tile([128, NCT * T], bf16, tag="xTall", name="xTall")
            xT = [xTall[:, i * T:(i + 1) * T] for i in range(NCT)]
            x8all = pp.tile([128, 8 * T], fp8, tag="x8all", name="x8all")
            x84 = x8all[:, :].rearrange("p (g ko t) -> p g ko t", g=4, ko=2)
            wqk8all = pp.tile([128, 8 * 512], fp8, tag="wqk8all", name="wqk8all")
            wqk84 = wqk8all[:, :].rearrange("p (g ko m) -> p g ko m", g=4, ko=2)
            wvall = pp.tile([128, NCT * 256], bf16, tag="wvall", name="wvall")
            wv = [wvall[:, i * 256:(i + 1) * 256] for i in range(NCT)]
            wpall = pp.tile([128, 2 * C], bf16, tag="wpall", name="wpall")
            wp = [wpall[:, p * C:(p + 1) * C] for p in range(2)]
            QTp = [pp.tile([128, T], bf16, tag=f"QT{p}", name=f"QT{p}")
                   for p in range(2)]
            KTp = [pp.tile([128, T], bf16, tag=f"KT{p}", name=f"KT{p}")
                   for p in range(2)]
            V2 = [pp.tile([128, NKT * VST], bf16, tag=f"V{p}", name=f"V{p}")
                  for p in range(2)]
            yT2 = [pp.tile([128, T], bf16, tag=f"yT{p}", name=f"yT{p}")
                   for p in range(2)]
            wseed = pp.tile([1, 64], bf16, tag="wseed", name="wseed")
            if with_bias:
                bqk = pp.tile([1, 512], bf16, tag="bqk", name="bqk")
                bv = pp.tile([1, 256], bf16, tag="bv", name="bv")
                ones_row = pp.tile([1, 512], bf16, tag="ones", name="ones")

            # ---- PE warm-up: tiny matmuls flip the HAM clock gate to
            # 2.4 GHz while the input DMAs stream in ----------------------
            nc.gpsimd.memset(wseed[:], 0.125)
            warm_ps = psp.tile([65, TQ], f32, tag="pv", bufs=2, name="warmps")
            for _ in range(72):
                nc.tensor.matmul(warm_ps[0:64, 0:64], lhsT=wseed[:],
                                 rhs=wseed[:], start=True, stop=True)

            # ---- batched input DMAs ------------------------------------
            # ~256-512KB chunks so transfers parallelize across queues while
            # keeping issue count low. First-needed data (wqk + xT strip 0)
            # leads on both HWDGE issue queues (Sync and Scalar).
            xT_d3 = xT_d.rearrange("(i p) t -> p i t", p=128)
            xTall3 = xTall[:, :].rearrange("p (i t) -> p i t", t=T)
            x8_d3 = x8_d.rearrange("p (q t) -> p q t", q=8)
            x8all3 = x8all[:, :].rearrange("p (q t) -> p q t", q=8)

            def x_chunk(eng, c, i0, i1):
                eng.dma_start(
                    xTall3[:, i0:i1, c * TQ:(c + 1) * TQ],
                    xT_d3[:, i0:i1, c * TQ:(c + 1) * TQ])

            # scalar queue: fp8 qk operands (strip 0 first), wp
            nc.scalar.dma_start(wqk8all[:], wqk8_d[:, :])
            for c in range(NSTRIP):
                nc.scalar.dma_start(x8all3[:, :, c * TQ:(c + 1) * TQ],
                                    x8_d3[:, :, c * TQ:(c + 1) * TQ])
            nc.scalar.dma_start(
                wpall[:, :].rearrange("p (i m) -> p i m", m=C),
                wp_d.rearrange("(i p) m -> p i m", p=128))
            # sync queue: bf16 xT strips (for the V matmuls), wv
            for i0 in range(0, 8, 2):
                x_chunk(nc.sync, 0, i0, i0 + 2)
            nc.sync.dma_start(
                wvall[:, :].rearrange("p (i m) -> p i m", m=256),
                wv_d.rearrange("(i p) m -> p i m", p=128))
            for i0 in range(0, 8, 4):
                x_chunk(nc.sync, 1, i0, i0 + 4)
            for i0 in range(0, 8, 4):
                x_chunk(nc.sync, 2, i0, i0 + 4)
            for i0 in range(0, 8, 4):
                x_chunk(nc.sync, 3, i0, i0 + 4)
            if with_bias:
                nc.scalar.dma_start(bqk[:], bqk_d[:, :])
                nc.scalar.dma_start(bv[:], bv_d[:, :])
                nc.gpsimd.memset(ones_row[:], 1.0)
            # ones columns for the softmax denominator: memset the whole V2
            # tiles to 1.0 once; the V columns get overwritten by v_steps.
            for p in range(2):
                nc.gpsimd.memset(V2[p][:], 1.0)

            nbias = 1 if with_bias else 0
            P_store = {}

            def A_steps(pair, qt):
                """Step list: each step emits one complete psum group."""
                steps = []

                def qk_step(mt):
                    def f():
                        ps = psp.tile([128, TQ], f32, tag="big", bufs=2, name="psA")
                        for g in range(4):
                            nc.tensor.matmul(
                                ps[:],
                                lhsT=wqk84[:, g, :, mt * 128:(mt + 1) * 128],
                                rhs=x84[:, g, :, qt * TQ:(qt + 1) * TQ],
                                start=(g == 0), stop=(g == 3 and not with_bias),
                                perf_mode=DoubleRow)
                        if with_bias:
                            nc.tensor.matmul(
                                ps[:], lhsT=bqk[0:1, mt * 128:(mt + 1) * 128],
                                rhs=ones_row[0:1, 0:TQ], start=False, stop=True)
                        dst = QTp[pair] if mt < 2 else KTp[pair]
                        nc.vector.tensor_copy(dst[:, qt * TQ:(qt + 1) * TQ], ps[:])
                    return f

                def v_step(kt):
                    def f():
                        psv = psp.tile([128, 256], f32, tag="big", bufs=2, name="psVt")
                        for ci in range(NCT):
                            nc.tensor.matmul(
                                psv[:],
                                lhsT=xT[ci][:, kt * 128:(kt + 1) * 128],
                                rhs=wv[ci][:, :],
                                start=(ci == 0), stop=(ci == NCT + nbias - 1))
                        if with_bias:
                            nc.tensor.matmul(
                                psv[:], lhsT=ones_row[0:1, 0:128], rhs=bv[0:1, :],
                                start=False, stop=True)
                        for p in range(2):
                            s3 = psv[:, p * 128:(p + 1) * 128] \
                                .rearrange("q (a b) -> q a b", b=64)
                            d3 = V2[p][:, kt * VST: kt * VST + VST] \
                                .rearrange("q (a b) -> q a b", b=68)[:, :, 0:64]
                            nc.vector.tensor_copy(d3, s3)
                    return f

                for mt in (pair, 2 + pair):
                    steps.append(qk_step(mt))
                if pair == 1:
                    for kt in range(4 * qt, 4 * qt + 4):
                        steps.append(v_step(kt))
                return steps

            def S_steps(pair, qt):
                """One step per k-tile: paired QK matmuls (disjoint PE row
                groups) + the two exps."""
                nk = 4 * (qt + 1)

                def kt_step(kt):
                    def f():
                        ps = psp.tile([128, 2 * TQ], f32, tag="S", bufs=2, name="psS")
                        m = kt - 4 * qt
                        off = max(0, m) * 128
                        for hh in range(2):
                            nc.tensor.matmul(
                                ps[:, hh * TQ + off:(hh + 1) * TQ],
                                lhsT=KTp[pair][hh * 64:(hh + 1) * 64,
                                               kt * 128:(kt + 1) * 128],
                                rhs=QTp[pair][hh * 64:(hh + 1) * 64,
                                              qt * TQ + off:(qt + 1) * TQ],
                                start=True, stop=True)
                        Pt = pP.tile([128, 2 * TQ], bf16, tag="P", bufs=32, name="Pt")
                        if m < 0:    # one exp across both heads' banks
                            nc.scalar.activation(Pt[:, :], ps[:, :], Exp, scale=0.125 / (WS * WS))
                        else:        # one 3D-AP exp covering both heads
                            nc.scalar.activation(
                                Pt[:, :].rearrange("k (h q) -> k h q", h=2)
                                        [:, :, off:TQ],
                                ps[:, :].rearrange("k (h q) -> k h q", h=2)
                                        [:, :, off:TQ],
                                Exp, scale=0.125 / (WS * WS))
                        for hh in range(2):
                            h = 2 * pair + hh
                            if m >= 0:   # diagonal block: zero k > q (gpsimd)
                                sl = Pt[:, hh * TQ + off: hh * TQ + off + 128]
                                nc.gpsimd.affine_select(
                                    out=sl, in_=sl,
                                    compare_op=mybir.AluOpType.is_ge,
                                    fill=0.0, base=0, pattern=[[1, 128]],
                                    channel_multiplier=-1)
                            P_store[(h, qt, kt)] = Pt[:, hh * TQ:(hh + 1) * TQ]
                    return f

                return [kt_step(kt) for kt in range(nk)]

            def PV_steps(pair, qt):
                """One step per head: PV accumulation (V stationary), then
                1/denom on DVE, partition-broadcast on gpsimd, multiply."""
                nk = 4 * (qt + 1)

                def head_step(hh):
                    def f():
                        h = 2 * pair + hh
                        psy = psp.tile([65, TQ], f32, tag="pv", bufs=2, name="psy")
                        for kt in range(nk):
                            off = max(0, kt - 4 * qt) * 128
                            nc.tensor.matmul(
                                psy[:, off:TQ],
                                lhsT=V2[pair][:, kt * VST + hh * 68:
                                              kt * VST + hh * 68 + 65],
                                rhs=P_store[(h, qt, kt)][:, off:TQ],
                                start=(kt == 0), stop=(kt == nk - 1))
                        drow = pRR.tile([1, TQ], f32, tag="rr", name="drow")
                        nc.vector.tensor_copy(drow[:], psy[64:65, :])
                        rfast = pRR.tile([1, TQ], f32, tag="rrb", name="rfast")
                        nc.vector.reciprocal_approx_fast(rfast[:], drow[:])
                        bcs = pBC.tile([64, TQ], f32, tag="bc", name="bcs")
                        nc.gpsimd.partition_broadcast(bcs[:], rfast[0:1, :],
                                                      channels=64)
                        nc.vector.tensor_mul(
                            yT2[pair][hh * 64:(hh + 1) * 64, qt * TQ:(qt + 1) * TQ],
                            psy[0:64, :], bcs[:])
                        if hh == 1:
                            for kt in range(nk):
                                del P_store[(2 * pair, qt, kt)]
                                del P_store[(2 * pair + 1, qt, kt)]
                    return f

                return [head_step(0), head_step(1)]

            def PJ_steps(qt):
                yOut = pO.tile([128, 8 * TQ], bf16, tag="yout", bufs=1,
                               name=f"yOut{qt}")

                def co_step(co):
                    def f():
                        pso = psp.tile([128, TQ], f32, tag="big", bufs=2, name="psO")
                        for pair in range(2):
                            nc.tensor.matmul(
                                pso[:],
                                lhsT=wp[pair][:, co * 128:(co + 1) * 128],
                                rhs=yT2[pair][:, qt * TQ:(qt + 1) * TQ],
                                start=(pair == 0), stop=(pair == 1))
                        nc.vector.tensor_copy(
                            yOut[:, co * TQ:(co + 1) * TQ], pso[:])
                        if co == 7:
                            nc.sync.dma_start(
                                out_d.rearrange("(i p) t -> p i t", p=128)
                                     [:, :, qt * TQ:(qt + 1) * TQ],
                                yOut[:, :].rearrange("p (i t) -> p i t", t=TQ))
                    return f

                return [co_step(co) for co in range(8)]

            def weave(s_list, others):
                """Interleave `others` proportionally between S k-tile steps."""
                if not s_list:
                    for f in others:
                        f()
                    return
                r = len(others) / len(s_list)
                acc, oi = 0.5, 0
                for f in s_list:
                    f()
                    acc += r
                    while acc >= 1.0 and oi < len(others):
                        others[oi]()
                        oi += 1
                        acc -= 1.0
                while oi < len(others):
                    others[oi]()
                    oi += 1

            # ---- software-pipelined, finely woven emission order -------
            # A steps run as early as their xT strip allows; each PV phase
            # follows its S phase one weave-line later so P tiles die fast
            # and the last S phase (pair 1, qt 3) overlaps PV(0,3)+PJ(2).
            for f in A_steps(0, 0):
                f()
            weave(S_steps(0, 0), A_steps(1, 0))
            weave(S_steps(1, 0), A_steps(0, 1) + PV_steps(0, 0))
            weave(S_steps(0, 1), A_steps(1, 1) + PV_steps(1, 0))
            weave(S_steps(1, 1), A_steps(0, 2) + PV_steps(0, 1) + PJ_steps(0))
            weave(S_steps(0, 2), A_steps(1, 2) + PV_steps(1, 1))
            weave(S_steps(1, 2), A_steps(0, 3) + PV_steps(0, 2) + PJ_steps(1))
            weave(S_steps(0, 3), A_steps(1, 3) + PV_steps(1, 2))
            weave(S_steps(1, 3), PV_steps(0, 3) + PJ_steps(2))
            weave([], PV_steps(1, 3) + PJ_steps(3))

    nc.compile()
    return nc


def _get_nc(with_bias: bool):
    key = ("nc", with_bias)
    if key not in _CACHE:
        _ensure_runtime()
        _CACHE[key] = _build(with_bias)
    return _CACHE[key]


def _shard_inputs(x, w_qkv, b_qkv, w_proj, with_bias):
    """Build the 8 per-core input maps (bf16)."""
    in_maps = []
    for core in range(N_CORES):
        b, g = core // G, core % G
        hs = [g * HPG + i for i in range(HPG)]
        q_cols = [w_qkv[:, h * DH:(h + 1) * DH] for h in hs]
        k_cols = [w_qkv[:, C + h * DH: C + (h + 1) * DH] for h in hs]
        v_cols = [w_qkv[:, 2 * C + h * DH: 2 * C + (h + 1) * DH] for h in hs]
        xt = np.ascontiguousarray(x[b].T)
        wqk = np.concatenate(q_cols + k_cols, axis=1) * WS

        def pack_dr(a):
            """[1024, n] -> DoubleRow fp8 layout [ki=128, g*ko*n]."""
            n = a.shape[1]
            return np.ascontiguousarray(
                a.reshape(4, 2, 128, n).transpose(2, 0, 1, 3)
                 .reshape(128, 8 * n)).astype(F8)

        m = {
            "xT": xt.astype(BF),
            "x8": pack_dr(xt),
            "wqk8": pack_dr(wqk),
            "wv": np.concatenate(v_cols, axis=1).astype(BF),
            "wp": np.concatenate(
                [w_proj[h * DH:(h + 1) * DH, :] for h in hs], axis=0).astype(BF),
        }
        if with_bias:
            bq = [b_qkv[h * DH:(h + 1) * DH] for h in hs]
            bk = [b_qkv[C + h * DH: C + (h + 1) * DH] for h in hs]
            bvs = [b_qkv[2 * C + h * DH: 2 * C + (h + 1) * DH] for h in hs]
            m["bqk"] = (np.concatenate(bq + bk) * WS)[None, :].astype(BF)
            m["bv"] = np.concatenate(bvs)[None, :].astype(BF)
        in_maps.append(m)
    return in_maps


def run_on_device(x, w_qkv, b_qkv, w_proj, b_proj, trace=False, trace_kwargs=None):
    """Returns (output [B,T,C] float32, BassKernelResults)."""
    x = np.asarray(x, np.float32)
    w_qkv = np.asarray(w_qkv, np.float32)
    b_qkv = np.asarray(b_qkv, np.float32)
    w_proj = np.asarray(w_proj, np.float32)
    b_proj = np.asarray(b_proj, np.float32)

    with_bias = bool(np.any(b_qkv))
    nc = _get_nc(with_bias)
    in_maps = _shard_inputs(x, w_qkv, b_qkv, w_proj, with_bias)

    from concourse.bass_utils import run_bass_kernel_spmd
    res = run_bass_kernel_spmd(nc, in_maps, core_ids=list(range(N_CORES)),
                               trace=trace, **(trace_kwargs or {}))

    out = np.zeros((B, T, C), np.float64)
    for core in range(N_CORES):
        b = core // G
        out[b] += res.results[core]["outT"].T.astype(np.float64)
    out += b_proj.astype(np.float64)[None, None, :]
    return out.astype(np.float32), res


def kernel(x, w_qkv, b_qkv, w_proj, b_proj):
    out, _ = run_on_device(x, w_qkv, b_qkv, w_proj, b_proj)
    return out
